# revision 53
# baseline (speedup 1.0000x reference)
"""Trainium2 Bass kernel for nn_LocalitySelfAttention.

The module's attention scores get +1e9 added on the diagonal before the
softmax (torch's ``attn - diag(-1e9)``).  QK^T scores for randn inputs are
O(1), so every softmax row is an exact fp32 one-hot at the diagonal and
``attn @ v == v`` bit-exactly.  The whole module therefore reduces to

    out = x @ Wv.T @ w_proj.T + b_proj,      Wv = w_qkv[512:768]

which is a memory-bound GEMM.  The kernel shards the 8192 (B*N) rows across
the 8 NeuronCores (1024 rows each).  Each core:

  1. folds W2T[k,p] = sum_vd Wv[vd,k] * w_proj[p,vd] on the TensorEngine,
  2. computes out[n,p] = sum_k xT[k,n] * W2T[k,p] + b[p] as 8 PSUM tiles;
     the bias-add happens during the PSUM->SBUF copy on the DVE, emitting
     bf16 (the rounding is done on-device; the host only zero-extends
     bf16->f32, which is exact), halving both the copy time and the
     output HBM traffic.

All matmul operands are typed float32r end-to-end (DRAM + SBUF), which the
PE streams at half fp32's cycles-per-row; the bytes are plain fp32 and the
PSUM accumulation stays fp32 (rel err ~2e-3 vs 2e-2 tolerance).

Measured HW model this is built around:
  - exec_time = last-output-byte time + fixed overhead: the ~6us NEFF
    start is excluded by the profiler's first-useful-instruction window
    and an ~8.5us finalization tail is constant, so everything aims at
    finishing the last output DMA byte early.  First DMA bytes land a
    fixed ~2.7us after the post-barrier triggers, and the 8-core input
    phase runs at the chip HBM roofline, so the input stream itself is
    the floor.
  - dma_start runs at ~5ns/descriptor on the issuing engine and both
    HWDGE rings (SP, Act) feed the same 16 HW queues in descriptor
    ARRIVAL order, so transfers use >=2KB lines and issue order is
    arranged as: weights -> bias -> first x half (SP), then second x
    half (Act, gated on the weights' completion by a tiny Act read so
    it cannot starve the fold); outputs alternate across both rings.
  - x chunks each get their own SBUF tile (a shared buffer would
    serialize a chunk's DMA behind every reader of the previous chunk);
    the kc0/kc1 planes are separate DMAs, and the second 512 columns
    arrive as quarters so only two row-tiles of work remain after the
    final chunk's completion semaphore (which itself lands ~1-1.5us
    after the data: a DMA's 16 queue-shard completions spread out).
  - a 128-descriptor stride-0 broadcast DMA crawls (~75 B/ns) and blocks
    queue FIFOs, so the bias arrives as ONE descriptor and is broadcast
    across partitions by a one-time ones x bias matmul on the PE.

The host only moves bytes: it transposes x, packs the weight block, and
unpermutes/widens the per-core output blocks (layout + zero-extension
only, no arithmetic).
"""

import os
import sys

import numpy as np

if "/opt/trn_rl_repo" not in sys.path:
    sys.path.insert(0, "/opt/trn_rl_repo")

B, N, C = 2, 4096, 256
ROWS = B * N              # 8192
NCORES = 8
RPC = ROWS // NCORES      # 1024 rows per core
NT = RPC // 128           # 8 row-tiles of 128 per core
OUTTILES = int(os.environ.get("K_OUTTILES", "2"))  # tiles per output DMA

USE_F32R = os.environ.get("K_F32R", "1") == "1"
OUT_BF16 = os.environ.get("K_OBF16", "1") == "1"
NWARM = int(os.environ.get("K_NWARM", "0"))   # PE clock-ramp matmuls

_cache = {}


def _build():
    """Build + compile the per-core Bass program (same program, SPMD)."""
    import concourse.bacc as bacc
    import concourse.bass as bass
    import concourse.mybir as mybir
    import concourse.tile as tile

    f32 = mybir.dt.float32
    mm_dt = mybir.dt.float32r if USE_F32R else f32
    out_dt = mybir.dt.bfloat16 if OUT_BF16 else f32

    nc = bacc.Bacc(
        "TRN2",
        target_bir_lowering=False,
        debug=False,
        num_devices=NCORES,
    )

    # All matmul inputs are typed f32r in DRAM too: the BIR verifier
    # requires every producer feeding an FP32r matmult to emit f32r, and
    # a DMA from an f32r DRAM tensor satisfies it (bytes are plain fp32).
    xt_d = nc.dram_tensor("xt", [C, RPC], mm_dt, kind="ExternalInput")
    wb_d = nc.dram_tensor("wb", [128, 4 * C], mm_dt, kind="ExternalInput")
    b_d = nc.dram_tensor("b", [C], f32, kind="ExternalInput")
    # output laid out [p, t, m] so multi-tile DMAs get fat contiguous lines;
    # the host undoes the (t p) permutation
    out_d = nc.dram_tensor("out", [128, NT * C], out_dt, kind="ExternalOutput")

    xt = xt_d.ap()
    wb = wb_d.ap()
    b = b_d.ap()
    out = out_d.ap()

    with tile.TileContext(nc) as tc:
        with (
            tc.tile_pool(name="const", bufs=1) as cp,
            tc.tile_pool(name="psw", bufs=2, space="PSUM") as psw,
            tc.tile_pool(name="pso", bufs=6, space="PSUM") as pso,
        ):
            # Both HWDGE rings (SP and Act) feed the SAME 16 HW queues in
            # descriptor-ARRIVAL order, so completion order is controlled
            # entirely by when each engine writes its descriptors.  Wanted
            # order: wb (fold) -> early x chunks -> late x chunks.

            # 64-byte ring-warmup DMAs, first thing on both HWDGE rings: if
            # the ~2.7us doorbell-to-first-byte latency is a one-time ring
            # setup cost, pay it on 64 bytes while the real descriptors are
            # still being written
            b_row = b.rearrange("(o c) -> o c", o=1)
            dwarm0 = cp.tile([1, 16], f32)
            nc.sync.dma_start(out=dwarm0, in_=b_row[0:1, 0:16])
            dwarm1 = cp.tile([1, 16], f32)
            nc.scalar.dma_start(out=dwarm1, in_=b_row[0:1, 0:16])

            # ---- weights first on SP: one DMA, 128 x 4KB lines ----
            # wb_sb[p, 0:2, k] = Wv[vdc*128+p, k]; [p, 2:4, q] = WprojT[vdc*128+p, q]
            wb_sb = cp.tile([128, 4, C], mm_dt)
            nc.sync.dma_start(out=wb_sb, in_=wb.rearrange("p (j k) -> p j k", j=4))

            # bias: ONE descriptor to a single partition (a 128-descriptor
            # stride-0 broadcast DMA crawls at ~75 B/ns and blocks every
            # queue FIFO behind it), then a one-time ones x bias matmul
            # broadcasts across partitions via the PE
            bias_sb = cp.tile([1, C], f32)
            nc.sync.dma_start(out=bias_sb, in_=b.rearrange("(o c) -> o c", o=1))
            ones_sb = cp.tile([1, 128], f32)
            nc.vector.memset(ones_sb, 1.0)

            # tiny Act-engine read of bias_sb: forces Act to wait until the
            # bias descriptor (queued right BEHIND all of wb's) completes
            # before issuing the late x chunks, so their descriptors arrive
            # after wb's and the fold is never starved behind x traffic.
            # bias's single-descriptor semaphore lands ~0.8us earlier than
            # wb's own 16-shard completion semaphore (shard completions
            # spread over ~1us), so it is the cheaper proxy for the same
            # ordering fact.
            wgate = cp.tile([1, 16], f32)
            nc.scalar.copy(wgate, bias_sb[0:1, 0:16])

            # ---- x^T slice, k-major [k=256, n=1024], chunked by column
            # group x kc.  Chunks complete in arrival order: the first 512
            # columns (tiles 0-3) issue on SP right behind wb; the second
            # half issues on Act behind the wb gate as QUARTERS (256 cols,
            # i.e. 2 tiles each) so after the very last chunk lands only
            # two tiles' matmuls remain.  Distinct tag per chunk — one
            # shared buffer would serialize chunk DMAs behind readers. ----
            xt_v = xt.rearrange("(kc p) n -> p kc n", p=128)
            # (engine, col_start, col_len) per chunk group; each group is
            # a kc0+kc1 pair of DMAs
            groups = [
                (nc.sync, 0, 512),
                (nc.scalar, 512, 256),
                (nc.scalar, 768, 256),
            ]
            tile_grp = [0, 0, 0, 0, 1, 1, 2, 2]   # row-tile -> chunk group
            xt_sbs = []      # [group][kc] -> tile [128, 1, col_len]
            for gi, (eng, c0, clen) in enumerate(groups):
                pair = []
                for kc in range(2):
                    xs = cp.tile([128, 1, clen], mm_dt, tag=f"xchunk{gi}_{kc}")
                    eng.dma_start(
                        out=xs,
                        in_=xt_v[:, kc:kc + 1, c0:c0 + clen],
                    )
                    pair.append(xs)
                xt_sbs.append(pair)

            # ---- PE warmup: the PE clock needs ~4.5us of sustained matmul
            # activity to ramp from the low pstate (213ns per 256-row f32r
            # matmul) to peak (~112ns).  Dummy matmuls fill the dead time
            # between engine start (~7us) and the weights landing (~11.5us)
            # so the fold and the main GEMM run at the ramped clock. ----
            if NWARM:
                warm_sb = cp.tile([128, 128], f32)
                nc.vector.memset(warm_sb, 0.0)
                warm_ps = psw.tile([128, C], f32, tag="w")
                for _ in range(NWARM):
                    nc.tensor.matmul(
                        warm_ps[:, 0:128], warm_sb, warm_sb,
                        start=True, stop=True,
                    )

            # ones x bias -> all-partition bias row block (PE broadcast)
            bias_bc = cp.tile([128, C], f32)
            ps_b = psw.tile([128, C], f32, tag="w")
            nc.tensor.matmul(ps_b, ones_sb, bias_sb, start=True, stop=True)
            nc.vector.tensor_copy(bias_bc, ps_b)

            # ---- fold W2T[k, p] = sum_vd Wv[vd, k] * wpt[vd, p] ----
            # (f32r consumers, so the PSUM->SBUF copy emits f32r)
            w2t_sb = cp.tile([128, 2, C], mm_dt)  # [p(k), kc, pcol]
            for kc in range(2):
                ps = psw.tile([128, C], f32, tag="w")
                for vdc in range(2):
                    nc.tensor.matmul(
                        ps,
                        wb_sb[:, vdc, kc * 128:(kc + 1) * 128],
                        wb_sb[:, 2 + vdc, :],
                        start=(vdc == 0),
                        stop=(vdc == 1),
                    )
                nc.vector.tensor_copy(w2t_sb[:, kc, :], ps)

            # ---- main GEMM: out[n, p] = sum_k xT[k, n] * W2T[k, p] + b[p] ----
            # all 8 output tiles live in one contiguous SBUF block so output
            # DMAs can cover several tiles with one fat line per partition
            ot_sb = cp.tile([128, NT, C], out_dt)
            # output DMA schedule: 2-tile chunks early (their drain hides
            # under remaining compute), single-tile chunks for the last two
            # tiles on ALTERNATE rings so the final drain is minimal
            out_sched = {1: (nc.scalar, 0), 3: (nc.sync, 2), 5: (nc.scalar, 4),
                         6: (nc.sync, 6), 7: (nc.scalar, 7)}
            for t in range(NT):
                gi = tile_grp[t]
                xk0, xk1 = xt_sbs[gi]
                tc_off = t * 128 - groups[gi][1]
                ps = pso.tile([128, C], f32)
                nc.tensor.matmul(
                    ps, xk0[:, 0, tc_off:tc_off + 128], w2t_sb[:, 0, :],
                    start=True, stop=False,
                )
                nc.tensor.matmul(
                    ps, xk1[:, 0, tc_off:tc_off + 128], w2t_sb[:, 1, :],
                    start=False, stop=True,
                )
                nc.vector.tensor_add(ot_sb[:, t, :], ps, bias_bc)
                if t in out_sched:
                    eng, t0 = out_sched[t]
                    eng.dma_start(
                        out=out[:, t0 * C:(t + 1) * C],
                        in_=ot_sb[:, t0:t + 1, :],
                    )

    nc.compile()
    return nc


def run_sharded(inputs, trace=False, trace_cores=None):
    """Shard inputs, run on the 8 NeuronCores, gather.  Returns
    (full_output, BassKernelResults)."""
    from concourse.bass_utils import run_bass_kernel_spmd

    x = np.ascontiguousarray(np.asarray(inputs["x"], dtype=np.float32))
    w_qkv = np.ascontiguousarray(np.asarray(inputs["w_qkv"], dtype=np.float32))
    w_proj = np.ascontiguousarray(np.asarray(inputs["w_proj"], dtype=np.float32))
    b_proj = np.ascontiguousarray(np.asarray(inputs["b_proj"], dtype=np.float32))

    if "nc" not in _cache:
        _cache["nc"] = _build()
    nc = _cache["nc"]

    # host-side layout marshaling only (no FLOPs)
    xT = np.ascontiguousarray(x.reshape(ROWS, C).T)          # [256, 8192]
    wv = w_qkv[2 * C:3 * C]                                  # [256, 256]
    wpt = w_proj.T                                           # [256, 256]
    # pack wv + wpt p-major: wb[p, j, :] for j in (wv kc0, wv kc1, wpt 0, wpt 1)
    wb = np.empty((128, 4, C), dtype=np.float32)
    wb[:, 0] = wv[0:128]
    wb[:, 1] = wv[128:256]
    wb[:, 2] = wpt[0:128]
    wb[:, 3] = wpt[128:256]
    wb = np.ascontiguousarray(wb.reshape(128, 4 * C))

    in_maps = [
        {
            "xt": np.ascontiguousarray(xT[:, c * RPC:(c + 1) * RPC]),
            "wb": wb,
            "b": b_proj,
        }
        for c in range(NCORES)
    ]

    res = run_bass_kernel_spmd(
        nc,
        in_maps,
        core_ids=list(range(NCORES)),
        trace=trace,
        trace_cores=trace_cores,
    )
    # device emits [p, t, m]; undo the (t p) row permutation and widen
    # bf16 -> f32 (exact zero-extension)
    blocks = []
    for c in range(NCORES):
        arr = np.asarray(res.results[c]["out"]).reshape(128, NT, C)
        blocks.append(
            np.ascontiguousarray(arr.transpose(1, 0, 2)).reshape(RPC, C).astype(np.float32)
        )
    out = np.concatenate(blocks, axis=0)  # [8192, 256]
    return out.reshape(B, N, C), res


def kernel(x, w_qkv, w_proj, b_proj, temperature):
    out, _ = run_sharded(
        {"x": x, "w_qkv": w_qkv, "w_proj": w_proj, "b_proj": b_proj}
    )
    return out


# revision 54
# speedup vs baseline: 1.0260x; 1.0260x over previous
"""Trainium2 Bass kernel for nn_LocalitySelfAttention.

The module's attention scores get +1e9 added on the diagonal before the
softmax (torch's ``attn - diag(-1e9)``).  QK^T scores for randn inputs are
O(1), so every softmax row is an exact fp32 one-hot at the diagonal and
``attn @ v == v`` bit-exactly.  The whole module therefore reduces to

    out = x @ Wv.T @ w_proj.T + b_proj,      Wv = w_qkv[512:768]

which is a memory-bound GEMM.  The kernel shards the 8192 (B*N) rows across
the 8 NeuronCores (1024 rows each).  Each core:

  1. folds W2T[k,p] = sum_vd Wv[vd,k] * w_proj[p,vd] on the TensorEngine,
  2. computes out[n,p] = sum_k xT[k,n] * W2T[k,p] + b[p] as 8 PSUM tiles;
     the bias-add happens during the PSUM->SBUF copy on the DVE, emitting
     bf16 (the rounding is done on-device; the host only zero-extends
     bf16->f32, which is exact), halving both the copy time and the
     output HBM traffic.

All matmul operands are typed float32r end-to-end (DRAM + SBUF), which the
PE streams at half fp32's cycles-per-row; the bytes are plain fp32 and the
PSUM accumulation stays fp32 (rel err ~2e-3 vs 2e-2 tolerance).

Measured HW model this is built around:
  - exec_time = last-output-byte time + fixed overhead: the ~6us NEFF
    start is excluded by the profiler's first-useful-instruction window
    and an ~8.5us finalization tail is constant, so everything aims at
    finishing the last output DMA byte early.  First DMA bytes land a
    fixed ~2.7us after the post-barrier triggers, and the 8-core input
    phase runs at the chip HBM roofline, so the input stream itself is
    the floor.
  - dma_start runs at ~5ns/descriptor on the issuing engine and both
    HWDGE rings (SP, Act) feed the same 16 HW queues in descriptor
    ARRIVAL order, so transfers use >=2KB lines and issue order is
    arranged as: weights -> bias -> first x half (SP), then second x
    half (Act, gated on the weights' completion by a tiny Act read so
    it cannot starve the fold); outputs alternate across both rings.
  - x chunks each get their own SBUF tile (a shared buffer would
    serialize a chunk's DMA behind every reader of the previous chunk);
    the kc0/kc1 planes are separate DMAs, and the second 512 columns
    arrive as quarters so only two row-tiles of work remain after the
    final chunk's completion semaphore (which itself lands ~1-1.5us
    after the data: a DMA's 16 queue-shard completions spread out).
  - a 128-descriptor stride-0 broadcast DMA crawls (~75 B/ns) and blocks
    queue FIFOs, so the bias arrives as ONE descriptor and is broadcast
    across partitions by a one-time ones x bias matmul on the PE.

The host only moves bytes: it transposes x, packs the weight block, and
unpermutes/widens the per-core output blocks (layout + zero-extension
only, no arithmetic).
"""

import os
import sys

import numpy as np

if "/opt/trn_rl_repo" not in sys.path:
    sys.path.insert(0, "/opt/trn_rl_repo")

B, N, C = 2, 4096, 256
ROWS = B * N              # 8192
NCORES = 8
RPC = ROWS // NCORES      # 1024 rows per core
NT = RPC // 128           # 8 row-tiles of 128 per core
OUTTILES = int(os.environ.get("K_OUTTILES", "2"))  # tiles per output DMA

USE_F32R = os.environ.get("K_F32R", "1") == "1"
OUT_BF16 = os.environ.get("K_OBF16", "1") == "1"
NWARM = int(os.environ.get("K_NWARM", "0"))   # PE clock-ramp matmuls

_cache = {}


def _build():
    """Build + compile the per-core Bass program (same program, SPMD)."""
    import concourse.bacc as bacc
    import concourse.bass as bass
    import concourse.mybir as mybir
    import concourse.tile as tile

    f32 = mybir.dt.float32
    mm_dt = mybir.dt.float32r if USE_F32R else f32
    out_dt = mybir.dt.bfloat16 if OUT_BF16 else f32

    nc = bacc.Bacc(
        "TRN2",
        target_bir_lowering=False,
        debug=False,
        num_devices=NCORES,
    )

    # All matmul inputs are typed f32r in DRAM too: the BIR verifier
    # requires every producer feeding an FP32r matmult to emit f32r, and
    # a DMA from an f32r DRAM tensor satisfies it (bytes are plain fp32).
    xt_d = nc.dram_tensor("xt", [C, RPC], mm_dt, kind="ExternalInput")
    wb_d = nc.dram_tensor("wb", [128, 4 * C], mm_dt, kind="ExternalInput")
    b_d = nc.dram_tensor("b", [C], f32, kind="ExternalInput")
    # output laid out [p, t, m] so multi-tile DMAs get fat contiguous lines;
    # the host undoes the (t p) permutation
    out_d = nc.dram_tensor("out", [128, NT * C], out_dt, kind="ExternalOutput")

    xt = xt_d.ap()
    wb = wb_d.ap()
    b = b_d.ap()
    out = out_d.ap()

    with tile.TileContext(nc) as tc:
        with (
            tc.tile_pool(name="const", bufs=1) as cp,
            tc.tile_pool(name="psw", bufs=2, space="PSUM") as psw,
            tc.tile_pool(name="pso", bufs=6, space="PSUM") as pso,
        ):
            # Both HWDGE rings (SP and Act) feed the SAME 16 HW queues in
            # descriptor-ARRIVAL order, so completion order is controlled
            # entirely by when each engine writes its descriptors.  Wanted
            # order: wb (fold) -> early x chunks -> late x chunks.

            # ---- weights first on SP: one DMA, 128 x 4KB lines ----
            # wb_sb[p, 0:2, k] = Wv[vdc*128+p, k]; [p, 2:4, q] = WprojT[vdc*128+p, q]
            wb_sb = cp.tile([128, 4, C], mm_dt)
            nc.sync.dma_start(out=wb_sb, in_=wb.rearrange("p (j k) -> p j k", j=4))

            # bias: ONE descriptor to a single partition (a 128-descriptor
            # stride-0 broadcast DMA crawls at ~75 B/ns and blocks every
            # queue FIFO behind it), then a one-time ones x bias matmul
            # broadcasts across partitions via the PE
            bias_sb = cp.tile([1, C], f32)
            nc.sync.dma_start(out=bias_sb, in_=b.rearrange("(o c) -> o c", o=1))
            ones_sb = cp.tile([1, 128], f32)
            nc.vector.memset(ones_sb, 1.0)

            # tiny Act-engine read of bias_sb: forces Act to wait until the
            # bias descriptor (queued right BEHIND all of wb's) completes
            # before issuing the late x chunks, so their descriptors arrive
            # after wb's and the fold is never starved behind x traffic.
            # bias's single-descriptor semaphore lands ~0.8us earlier than
            # wb's own 16-shard completion semaphore (shard completions
            # spread over ~1us), so it is the cheaper proxy for the same
            # ordering fact.
            wgate = cp.tile([1, 16], f32)
            nc.scalar.copy(wgate, bias_sb[0:1, 0:16])

            # ---- x^T slice, k-major [k=256, n=1024], chunked by column
            # group x kc.  Chunks complete in arrival order: the first 512
            # columns (tiles 0-3) issue on SP right behind wb; the second
            # half issues on Act behind the wb gate as QUARTERS (256 cols,
            # i.e. 2 tiles each) so after the very last chunk lands only
            # two tiles' matmuls remain.  Distinct tag per chunk — one
            # shared buffer would serialize chunk DMAs behind readers. ----
            xt_v = xt.rearrange("(kc p) n -> p kc n", p=128)
            # (engine, col_start, col_len) per chunk group; each group is
            # a kc0+kc1 pair of DMAs
            groups = [
                (nc.sync, 0, 512),
                (nc.scalar, 512, 256),
                (nc.scalar, 768, 256),
            ]
            tile_grp = [0, 0, 0, 0, 1, 1, 2, 2]   # row-tile -> chunk group
            xt_sbs = []      # [group][kc] -> tile [128, 1, col_len]
            for gi, (eng, c0, clen) in enumerate(groups):
                pair = []
                for kc in range(2):
                    xs = cp.tile([128, 1, clen], mm_dt, tag=f"xchunk{gi}_{kc}")
                    eng.dma_start(
                        out=xs,
                        in_=xt_v[:, kc:kc + 1, c0:c0 + clen],
                    )
                    pair.append(xs)
                xt_sbs.append(pair)

            # ---- PE warmup: the PE clock needs ~4.5us of sustained matmul
            # activity to ramp from the low pstate (213ns per 256-row f32r
            # matmul) to peak (~112ns).  Dummy matmuls fill the dead time
            # between engine start (~7us) and the weights landing (~11.5us)
            # so the fold and the main GEMM run at the ramped clock. ----
            if NWARM:
                warm_sb = cp.tile([128, 128], f32)
                nc.vector.memset(warm_sb, 0.0)
                warm_ps = psw.tile([128, C], f32, tag="w")
                for _ in range(NWARM):
                    nc.tensor.matmul(
                        warm_ps[:, 0:128], warm_sb, warm_sb,
                        start=True, stop=True,
                    )

            # ones x bias -> all-partition bias row block (PE broadcast)
            bias_bc = cp.tile([128, C], f32)
            ps_b = psw.tile([128, C], f32, tag="w")
            nc.tensor.matmul(ps_b, ones_sb, bias_sb, start=True, stop=True)
            nc.vector.tensor_copy(bias_bc, ps_b)

            # ---- fold W2T[k, p] = sum_vd Wv[vd, k] * wpt[vd, p] ----
            # (f32r consumers, so the PSUM->SBUF copy emits f32r)
            w2t_sb = cp.tile([128, 2, C], mm_dt)  # [p(k), kc, pcol]
            for kc in range(2):
                ps = psw.tile([128, C], f32, tag="w")
                for vdc in range(2):
                    nc.tensor.matmul(
                        ps,
                        wb_sb[:, vdc, kc * 128:(kc + 1) * 128],
                        wb_sb[:, 2 + vdc, :],
                        start=(vdc == 0),
                        stop=(vdc == 1),
                    )
                nc.vector.tensor_copy(w2t_sb[:, kc, :], ps)

            # ---- main GEMM: out[n, p] = sum_k xT[k, n] * W2T[k, p] + b[p] ----
            # all 8 output tiles live in one contiguous SBUF block so output
            # DMAs can cover several tiles with one fat line per partition
            ot_sb = cp.tile([128, NT, C], out_dt)
            # output DMA schedule: 2-tile chunks early (their drain hides
            # under remaining compute), single-tile chunks for the last two
            # tiles on ALTERNATE rings so the final drain is minimal
            out_sched = {1: (nc.scalar, 0), 3: (nc.sync, 2), 5: (nc.scalar, 4),
                         6: (nc.sync, 6), 7: (nc.scalar, 7)}
            for t in range(NT):
                gi = tile_grp[t]
                xk0, xk1 = xt_sbs[gi]
                tc_off = t * 128 - groups[gi][1]
                ps = pso.tile([128, C], f32)
                nc.tensor.matmul(
                    ps, xk0[:, 0, tc_off:tc_off + 128], w2t_sb[:, 0, :],
                    start=True, stop=False,
                )
                nc.tensor.matmul(
                    ps, xk1[:, 0, tc_off:tc_off + 128], w2t_sb[:, 1, :],
                    start=False, stop=True,
                )
                nc.vector.tensor_add(ot_sb[:, t, :], ps, bias_bc)
                if t in out_sched:
                    eng, t0 = out_sched[t]
                    eng.dma_start(
                        out=out[:, t0 * C:(t + 1) * C],
                        in_=ot_sb[:, t0:t + 1, :],
                    )

    nc.compile()
    return nc


def run_sharded(inputs, trace=False, trace_cores=None):
    """Shard inputs, run on the 8 NeuronCores, gather.  Returns
    (full_output, BassKernelResults)."""
    from concourse.bass_utils import run_bass_kernel_spmd

    x = np.ascontiguousarray(np.asarray(inputs["x"], dtype=np.float32))
    w_qkv = np.ascontiguousarray(np.asarray(inputs["w_qkv"], dtype=np.float32))
    w_proj = np.ascontiguousarray(np.asarray(inputs["w_proj"], dtype=np.float32))
    b_proj = np.ascontiguousarray(np.asarray(inputs["b_proj"], dtype=np.float32))

    if "nc" not in _cache:
        _cache["nc"] = _build()
    nc = _cache["nc"]

    # host-side layout marshaling only (no FLOPs)
    xT = np.ascontiguousarray(x.reshape(ROWS, C).T)          # [256, 8192]
    wv = w_qkv[2 * C:3 * C]                                  # [256, 256]
    wpt = w_proj.T                                           # [256, 256]
    # pack wv + wpt p-major: wb[p, j, :] for j in (wv kc0, wv kc1, wpt 0, wpt 1)
    wb = np.empty((128, 4, C), dtype=np.float32)
    wb[:, 0] = wv[0:128]
    wb[:, 1] = wv[128:256]
    wb[:, 2] = wpt[0:128]
    wb[:, 3] = wpt[128:256]
    wb = np.ascontiguousarray(wb.reshape(128, 4 * C))

    in_maps = [
        {
            "xt": np.ascontiguousarray(xT[:, c * RPC:(c + 1) * RPC]),
            "wb": wb,
            "b": b_proj,
        }
        for c in range(NCORES)
    ]

    res = run_bass_kernel_spmd(
        nc,
        in_maps,
        core_ids=list(range(NCORES)),
        trace=trace,
        trace_cores=trace_cores,
    )
    # device emits [p, t, m]; undo the (t p) row permutation and widen
    # bf16 -> f32 (exact zero-extension)
    blocks = []
    for c in range(NCORES):
        arr = np.asarray(res.results[c]["out"]).reshape(128, NT, C)
        blocks.append(
            np.ascontiguousarray(arr.transpose(1, 0, 2)).reshape(RPC, C).astype(np.float32)
        )
    out = np.concatenate(blocks, axis=0)  # [8192, 256]
    return out.reshape(B, N, C), res


def kernel(x, w_qkv, w_proj, b_proj, temperature):
    out, _ = run_sharded(
        {"x": x, "w_qkv": w_qkv, "w_proj": w_proj, "b_proj": b_proj}
    )
    return out


# revision 55
# speedup vs baseline: 1.0336x; 1.0074x over previous
"""Trainium2 Bass kernel for nn_LocalitySelfAttention.

The module's attention scores get +1e9 added on the diagonal before the
softmax (torch's ``attn - diag(-1e9)``).  QK^T scores for randn inputs are
O(1), so every softmax row is an exact fp32 one-hot at the diagonal and
``attn @ v == v`` bit-exactly.  The whole module therefore reduces to

    out = x @ Wv.T @ w_proj.T + b_proj,      Wv = w_qkv[512:768]

which is a memory-bound GEMM.  The kernel shards the 8192 (B*N) rows across
the 8 NeuronCores (1024 rows each).  Each core:

  1. folds W2T[k,p] = sum_vd Wv[vd,k] * w_proj[p,vd] on the TensorEngine,
  2. computes out[n,p] = sum_k xT[k,n] * W2T[k,p] + b[p] as 8 PSUM tiles;
     the bias-add happens during the PSUM->SBUF copy on the DVE, emitting
     bf16 (the rounding is done on-device; the host only zero-extends
     bf16->f32, which is exact), halving both the copy time and the
     output HBM traffic.

All matmul operands are typed float32r end-to-end (DRAM + SBUF), which the
PE streams at half fp32's cycles-per-row; the bytes are plain fp32 and the
PSUM accumulation stays fp32 (rel err ~2e-3 vs 2e-2 tolerance).

Measured HW model this is built around:
  - exec_time = last-output-byte time + fixed overhead: the ~6us NEFF
    start is excluded by the profiler's first-useful-instruction window
    and an ~8.5us finalization tail is constant, so everything aims at
    finishing the last output DMA byte early.  First DMA bytes land a
    fixed ~2.7us after the post-barrier triggers, and the 8-core input
    phase runs at the chip HBM roofline, so the input stream itself is
    the floor.
  - dma_start runs at ~5ns/descriptor on the issuing engine and both
    HWDGE rings (SP, Act) feed the same 16 HW queues in descriptor
    ARRIVAL order, so transfers use >=2KB lines and issue order is
    arranged as: weights -> bias -> first x half (SP), then second x
    half (Act, gated on the weights' completion by a tiny Act read so
    it cannot starve the fold); outputs alternate across both rings.
  - x chunks each get their own SBUF tile (a shared buffer would
    serialize a chunk's DMA behind every reader of the previous chunk);
    the kc0/kc1 planes are separate DMAs, and the second 512 columns
    arrive as quarters so only two row-tiles of work remain after the
    final chunk's completion semaphore (which itself lands ~1-1.5us
    after the data: a DMA's 16 queue-shard completions spread out).
  - a 128-descriptor stride-0 broadcast DMA crawls (~75 B/ns) and blocks
    queue FIFOs, so the bias arrives as ONE descriptor and is broadcast
    across partitions by a one-time ones x bias matmul on the PE.

The host only moves bytes: it transposes x, packs the weight block, and
unpermutes/widens the per-core output blocks (layout + zero-extension
only, no arithmetic).
"""

import os
import sys

import numpy as np

if "/opt/trn_rl_repo" not in sys.path:
    sys.path.insert(0, "/opt/trn_rl_repo")

B, N, C = 2, 4096, 256
ROWS = B * N              # 8192
NCORES = 8
RPC = ROWS // NCORES      # 1024 rows per core
NT = RPC // 128           # 8 row-tiles of 128 per core
OUTTILES = int(os.environ.get("K_OUTTILES", "2"))  # tiles per output DMA

USE_F32R = os.environ.get("K_F32R", "1") == "1"
OUT_BF16 = os.environ.get("K_OBF16", "1") == "1"
NWARM = int(os.environ.get("K_NWARM", "0"))   # PE clock-ramp matmuls

_cache = {}


def _build():
    """Build + compile the per-core Bass program (same program, SPMD)."""
    import concourse.bacc as bacc
    import concourse.bass as bass
    import concourse.mybir as mybir
    import concourse.tile as tile

    f32 = mybir.dt.float32
    mm_dt = mybir.dt.float32r if USE_F32R else f32
    out_dt = mybir.dt.bfloat16 if OUT_BF16 else f32

    nc = bacc.Bacc(
        "TRN2",
        target_bir_lowering=False,
        debug=False,
        num_devices=NCORES,
    )

    # All matmul inputs are typed f32r in DRAM too: the BIR verifier
    # requires every producer feeding an FP32r matmult to emit f32r, and
    # a DMA from an f32r DRAM tensor satisfies it (bytes are plain fp32).
    xt_d = nc.dram_tensor("xt", [C, RPC], mm_dt, kind="ExternalInput")
    wb_d = nc.dram_tensor("wb", [128, 4 * C], mm_dt, kind="ExternalInput")
    b_d = nc.dram_tensor("b", [C], f32, kind="ExternalInput")
    # output laid out [p, t, m] so multi-tile DMAs get fat contiguous lines;
    # the host undoes the (t p) permutation
    out_d = nc.dram_tensor("out", [128, NT * C], out_dt, kind="ExternalOutput")

    xt = xt_d.ap()
    wb = wb_d.ap()
    b = b_d.ap()
    out = out_d.ap()

    with tile.TileContext(nc) as tc:
        with (
            tc.tile_pool(name="const", bufs=1) as cp,
            tc.tile_pool(name="psw", bufs=3, space="PSUM") as psw,
            tc.tile_pool(name="pso", bufs=5, space="PSUM") as pso,
        ):
            # Both HWDGE rings (SP and Act) feed the SAME 16 HW queues in
            # descriptor-ARRIVAL order, so completion order is controlled
            # entirely by when each engine writes its descriptors.  Wanted
            # order: wb (fold) -> early x chunks -> late x chunks.

            # ---- weights first on SP: one DMA, 128 x 4KB lines ----
            # wb_sb[p, 0:2, k] = Wv[vdc*128+p, k]; [p, 2:4, q] = WprojT[vdc*128+p, q]
            wb_sb = cp.tile([128, 4, C], mm_dt)
            nc.sync.dma_start(out=wb_sb, in_=wb.rearrange("p (j k) -> p j k", j=4))

            # bias: ONE descriptor to a single partition (a 128-descriptor
            # stride-0 broadcast DMA crawls at ~75 B/ns and blocks every
            # queue FIFO behind it), then a one-time ones x bias matmul
            # broadcasts across partitions via the PE
            bias_sb = cp.tile([1, C], f32)
            nc.sync.dma_start(out=bias_sb, in_=b.rearrange("(o c) -> o c", o=1))
            ones_sb = cp.tile([1, 128], f32)
            nc.vector.memset(ones_sb, 1.0)

            # tiny Act-engine read of bias_sb: forces Act to wait until the
            # bias descriptor (queued right BEHIND all of wb's) completes
            # before issuing the late x chunks, so their descriptors arrive
            # after wb's and the fold is never starved behind x traffic.
            # bias's single-descriptor semaphore lands ~0.8us earlier than
            # wb's own 16-shard completion semaphore (shard completions
            # spread over ~1us), so it is the cheaper proxy for the same
            # ordering fact.
            wgate = cp.tile([1, 16], f32)
            nc.scalar.copy(wgate, bias_sb[0:1, 0:16])

            # ---- x^T slice, k-major [k=256, n=1024], chunked by column
            # group x kc.  Chunks complete in arrival order: the first 512
            # columns (tiles 0-3) issue on SP right behind wb; the second
            # half issues on Act behind the wb gate as QUARTERS (256 cols,
            # i.e. 2 tiles each) so after the very last chunk lands only
            # two tiles' matmuls remain.  Distinct tag per chunk — one
            # shared buffer would serialize chunk DMAs behind readers. ----
            xt_v = xt.rearrange("(kc p) n -> p kc n", p=128)
            # (engine, col_start, col_len) per chunk group; each group is
            # a kc0+kc1 pair of DMAs
            groups = [
                (nc.sync, 0, 512),
                (nc.scalar, 512, 256),
                (nc.scalar, 768, 256),
            ]
            tile_grp = [0, 0, 0, 0, 1, 1, 2, 2]   # row-tile -> chunk group
            xt_sbs = []      # [group][kc] -> tile [128, 1, col_len]
            for gi, (eng, c0, clen) in enumerate(groups):
                pair = []
                for kc in range(2):
                    xs = cp.tile([128, 1, clen], mm_dt, tag=f"xchunk{gi}_{kc}")
                    eng.dma_start(
                        out=xs,
                        in_=xt_v[:, kc:kc + 1, c0:c0 + clen],
                    )
                    pair.append(xs)
                xt_sbs.append(pair)

            # ---- PE warmup: the PE clock needs ~4.5us of sustained matmul
            # activity to ramp from the low pstate (213ns per 256-row f32r
            # matmul) to peak (~112ns).  Dummy matmuls fill the dead time
            # between engine start (~7us) and the weights landing (~11.5us)
            # so the fold and the main GEMM run at the ramped clock. ----
            if NWARM:
                warm_sb = cp.tile([128, 128], f32)
                nc.vector.memset(warm_sb, 0.0)
                warm_ps = psw.tile([128, C], f32, tag="w")
                for _ in range(NWARM):
                    nc.tensor.matmul(
                        warm_ps[:, 0:128], warm_sb, warm_sb,
                        start=True, stop=True,
                    )

            # ones x bias -> all-partition bias row block (PE broadcast)
            bias_bc = cp.tile([128, C], f32)
            ps_b = psw.tile([128, C], f32, tag="w")
            nc.tensor.matmul(ps_b, ones_sb, bias_sb, start=True, stop=True)
            nc.vector.tensor_copy(bias_bc, ps_b)

            # ---- fold W2T[k, p] = sum_vd Wv[vd, k] * wpt[vd, p] ----
            # (f32r consumers, so the PSUM->SBUF copy emits f32r)
            w2t_sb = cp.tile([128, 2, C], mm_dt)  # [p(k), kc, pcol]
            for kc in range(2):
                ps = psw.tile([128, C], f32, tag="w")
                for vdc in range(2):
                    nc.tensor.matmul(
                        ps,
                        wb_sb[:, vdc, kc * 128:(kc + 1) * 128],
                        wb_sb[:, 2 + vdc, :],
                        start=(vdc == 0),
                        stop=(vdc == 1),
                    )
                nc.vector.tensor_copy(w2t_sb[:, kc, :], ps)

            # ---- main GEMM: out[n, p] = sum_k xT[k, n] * W2T[k, p] + b[p] ----
            # all 8 output tiles live in one contiguous SBUF block so output
            # DMAs can cover several tiles with one fat line per partition
            ot_sb = cp.tile([128, NT, C], out_dt)
            # output DMA schedule: 2-tile chunks early (their drain hides
            # under remaining compute), single-tile chunks for the last two
            # tiles on ALTERNATE rings so the final drain is minimal
            out_sched = {1: (nc.scalar, 0), 3: (nc.sync, 2), 5: (nc.scalar, 4),
                         6: (nc.sync, 6), 7: (nc.scalar, 7)}
            for t in range(NT):
                gi = tile_grp[t]
                xk0, xk1 = xt_sbs[gi]
                tc_off = t * 128 - groups[gi][1]
                ps = pso.tile([128, C], f32)
                nc.tensor.matmul(
                    ps, xk0[:, 0, tc_off:tc_off + 128], w2t_sb[:, 0, :],
                    start=True, stop=False,
                )
                nc.tensor.matmul(
                    ps, xk1[:, 0, tc_off:tc_off + 128], w2t_sb[:, 1, :],
                    start=False, stop=True,
                )
                nc.vector.tensor_add(ot_sb[:, t, :], ps, bias_bc)
                if t in out_sched:
                    eng, t0 = out_sched[t]
                    eng.dma_start(
                        out=out[:, t0 * C:(t + 1) * C],
                        in_=ot_sb[:, t0:t + 1, :],
                    )

    nc.compile()
    return nc


def run_sharded(inputs, trace=False, trace_cores=None):
    """Shard inputs, run on the 8 NeuronCores, gather.  Returns
    (full_output, BassKernelResults)."""
    from concourse.bass_utils import run_bass_kernel_spmd

    x = np.ascontiguousarray(np.asarray(inputs["x"], dtype=np.float32))
    w_qkv = np.ascontiguousarray(np.asarray(inputs["w_qkv"], dtype=np.float32))
    w_proj = np.ascontiguousarray(np.asarray(inputs["w_proj"], dtype=np.float32))
    b_proj = np.ascontiguousarray(np.asarray(inputs["b_proj"], dtype=np.float32))

    if "nc" not in _cache:
        _cache["nc"] = _build()
    nc = _cache["nc"]

    # host-side layout marshaling only (no FLOPs)
    xT = np.ascontiguousarray(x.reshape(ROWS, C).T)          # [256, 8192]
    wv = w_qkv[2 * C:3 * C]                                  # [256, 256]
    wpt = w_proj.T                                           # [256, 256]
    # pack wv + wpt p-major: wb[p, j, :] for j in (wv kc0, wv kc1, wpt 0, wpt 1)
    wb = np.empty((128, 4, C), dtype=np.float32)
    wb[:, 0] = wv[0:128]
    wb[:, 1] = wv[128:256]
    wb[:, 2] = wpt[0:128]
    wb[:, 3] = wpt[128:256]
    wb = np.ascontiguousarray(wb.reshape(128, 4 * C))

    in_maps = [
        {
            "xt": np.ascontiguousarray(xT[:, c * RPC:(c + 1) * RPC]),
            "wb": wb,
            "b": b_proj,
        }
        for c in range(NCORES)
    ]

    res = run_bass_kernel_spmd(
        nc,
        in_maps,
        core_ids=list(range(NCORES)),
        trace=trace,
        trace_cores=trace_cores,
    )
    # device emits [p, t, m]; undo the (t p) row permutation and widen
    # bf16 -> f32 (exact zero-extension)
    blocks = []
    for c in range(NCORES):
        arr = np.asarray(res.results[c]["out"]).reshape(128, NT, C)
        blocks.append(
            np.ascontiguousarray(arr.transpose(1, 0, 2)).reshape(RPC, C).astype(np.float32)
        )
    out = np.concatenate(blocks, axis=0)  # [8192, 256]
    return out.reshape(B, N, C), res


def kernel(x, w_qkv, w_proj, b_proj, temperature):
    out, _ = run_sharded(
        {"x": x, "w_qkv": w_qkv, "w_proj": w_proj, "b_proj": b_proj}
    )
    return out


# revision 56
# speedup vs baseline: 1.0369x; 1.0032x over previous
"""Trainium2 Bass kernel for nn_LocalitySelfAttention.

The module's attention scores get +1e9 added on the diagonal before the
softmax (torch's ``attn - diag(-1e9)``).  QK^T scores for randn inputs are
O(1), so every softmax row is an exact fp32 one-hot at the diagonal and
``attn @ v == v`` bit-exactly.  The whole module therefore reduces to

    out = x @ Wv.T @ w_proj.T + b_proj,      Wv = w_qkv[512:768]

which is a memory-bound GEMM.  The kernel shards the 8192 (B*N) rows across
the 8 NeuronCores (1024 rows each).  Each core:

  1. folds W2T[k,p] = sum_vd Wv[vd,k] * w_proj[p,vd] on the TensorEngine,
  2. computes out[n,p] = sum_k xT[k,n] * W2T[k,p] + b[p] as 8 PSUM tiles;
     the bias-add happens during the PSUM->SBUF copy on the DVE, emitting
     bf16 (the rounding is done on-device; the host only zero-extends
     bf16->f32, which is exact), halving both the copy time and the
     output HBM traffic.

All matmul operands are typed float32r end-to-end (DRAM + SBUF), which the
PE streams at half fp32's cycles-per-row; the bytes are plain fp32 and the
PSUM accumulation stays fp32 (rel err ~2e-3 vs 2e-2 tolerance).

Measured HW model this is built around:
  - exec_time = last-output-byte time + fixed overhead: the ~6us NEFF
    start is excluded by the profiler's first-useful-instruction window
    and an ~8.5us finalization tail is constant, so everything aims at
    finishing the last output DMA byte early.  First DMA bytes land a
    fixed ~2.7us after the post-barrier triggers, and the 8-core input
    phase runs at the chip HBM roofline, so the input stream itself is
    the floor.
  - dma_start runs at ~5ns/descriptor on the issuing engine and both
    HWDGE rings (SP, Act) feed the same 16 HW queues in descriptor
    ARRIVAL order, so transfers use >=2KB lines and issue order is
    arranged as: weights -> bias -> first x half (SP), then second x
    half (Act, gated on the weights' completion by a tiny Act read so
    it cannot starve the fold); outputs alternate across both rings.
  - x chunks each get their own SBUF tile (a shared buffer would
    serialize a chunk's DMA behind every reader of the previous chunk);
    the kc0/kc1 planes are separate DMAs, and the second 512 columns
    arrive as quarters so only two row-tiles of work remain after the
    final chunk's completion semaphore (which itself lands ~1-1.5us
    after the data: a DMA's 16 queue-shard completions spread out).
  - a 128-descriptor stride-0 broadcast DMA crawls (~75 B/ns) and blocks
    queue FIFOs, so the bias arrives as ONE descriptor and is broadcast
    across partitions by a one-time ones x bias matmul on the PE.

The host only moves bytes: it transposes x, packs the weight block, and
unpermutes/widens the per-core output blocks (layout + zero-extension
only, no arithmetic).
"""

import os
import sys

import numpy as np

if "/opt/trn_rl_repo" not in sys.path:
    sys.path.insert(0, "/opt/trn_rl_repo")

B, N, C = 2, 4096, 256
ROWS = B * N              # 8192
NCORES = 8
RPC = ROWS // NCORES      # 1024 rows per core
NT = RPC // 128           # 8 row-tiles of 128 per core
OUTTILES = int(os.environ.get("K_OUTTILES", "2"))  # tiles per output DMA

USE_F32R = os.environ.get("K_F32R", "1") == "1"
OUT_BF16 = os.environ.get("K_OBF16", "1") == "1"
NWARM = int(os.environ.get("K_NWARM", "0"))   # PE clock-ramp matmuls

_cache = {}


def _build():
    """Build + compile the per-core Bass program (same program, SPMD)."""
    import concourse.bacc as bacc
    import concourse.bass as bass
    import concourse.mybir as mybir
    import concourse.tile as tile

    f32 = mybir.dt.float32
    mm_dt = mybir.dt.float32r if USE_F32R else f32
    out_dt = mybir.dt.bfloat16 if OUT_BF16 else f32

    nc = bacc.Bacc(
        "TRN2",
        target_bir_lowering=False,
        debug=False,
        num_devices=NCORES,
    )

    # All matmul inputs are typed f32r in DRAM too: the BIR verifier
    # requires every producer feeding an FP32r matmult to emit f32r, and
    # a DMA from an f32r DRAM tensor satisfies it (bytes are plain fp32).
    xt_d = nc.dram_tensor("xt", [C, RPC], mm_dt, kind="ExternalInput")
    wb_d = nc.dram_tensor("wb", [128, 4 * C], mm_dt, kind="ExternalInput")
    b_d = nc.dram_tensor("b", [C], f32, kind="ExternalInput")
    # output laid out [p, t, m] so multi-tile DMAs get fat contiguous lines;
    # the host undoes the (t p) permutation
    out_d = nc.dram_tensor("out", [128, NT * C], out_dt, kind="ExternalOutput")

    xt = xt_d.ap()
    wb = wb_d.ap()
    b = b_d.ap()
    out = out_d.ap()

    with tile.TileContext(nc) as tc:
        with (
            tc.tile_pool(name="const", bufs=1) as cp,
            tc.tile_pool(name="psw", bufs=3, space="PSUM") as psw,
            tc.tile_pool(name="pso", bufs=5, space="PSUM") as pso,
        ):
            # Both HWDGE rings (SP and Act) feed the SAME 16 HW queues in
            # descriptor-ARRIVAL order, so completion order is controlled
            # entirely by when each engine writes its descriptors.  Wanted
            # order: wb (fold) -> early x chunks -> late x chunks.

            # ---- weights first on SP: one DMA, 128 x 4KB lines ----
            # wb_sb[p, 0:2, k] = Wv[vdc*128+p, k]; [p, 2:4, q] = WprojT[vdc*128+p, q]
            wb_sb = cp.tile([128, 4, C], mm_dt)
            nc.sync.dma_start(out=wb_sb, in_=wb.rearrange("p (j k) -> p j k", j=4))

            # bias: ONE descriptor to a single partition (a 128-descriptor
            # stride-0 broadcast DMA crawls at ~75 B/ns and blocks every
            # queue FIFO behind it), then a one-time ones x bias matmul
            # broadcasts across partitions via the PE
            bias_sb = cp.tile([1, C], f32)
            nc.sync.dma_start(out=bias_sb, in_=b.rearrange("(o c) -> o c", o=1))
            ones_sb = cp.tile([1, 128], f32)
            nc.vector.memset(ones_sb, 1.0)

            # tiny Act-engine read of bias_sb: forces Act to wait until the
            # bias descriptor (queued right BEHIND all of wb's) completes
            # before issuing the late x chunks, so their descriptors arrive
            # after wb's and the fold is never starved behind x traffic.
            # bias's single-descriptor semaphore lands ~0.8us earlier than
            # wb's own 16-shard completion semaphore (shard completions
            # spread over ~1us), so it is the cheaper proxy for the same
            # ordering fact.
            wgate = cp.tile([1, 16], f32)
            nc.scalar.copy(wgate, bias_sb[0:1, 0:16])

            # ---- x^T slice, k-major [k=256, n=1024], chunked by column
            # group x kc.  Chunks complete in arrival order: the first 512
            # columns (tiles 0-3) issue on SP right behind wb; the second
            # half issues on Act behind the wb gate as QUARTERS (256 cols,
            # i.e. 2 tiles each) so after the very last chunk lands only
            # two tiles' matmuls remain.  Distinct tag per chunk — one
            # shared buffer would serialize chunk DMAs behind readers. ----
            xt_v = xt.rearrange("(kc p) n -> p kc n", p=128)
            # (engine, col_start, col_len) per chunk group; each group is
            # a kc0+kc1 pair of DMAs
            groups = [
                (nc.sync, 0, 512),
                (nc.scalar, 512, 256),
                (nc.scalar, 768, 256),
            ]
            tile_grp = [0, 0, 0, 0, 1, 1, 2, 2]   # row-tile -> chunk group
            xt_sbs = []      # [group][kc] -> tile [128, 1, col_len]
            for gi, (eng, c0, clen) in enumerate(groups):
                pair = []
                for kc in range(2):
                    xs = cp.tile([128, 1, clen], mm_dt, tag=f"xchunk{gi}_{kc}")
                    eng.dma_start(
                        out=xs,
                        in_=xt_v[:, kc:kc + 1, c0:c0 + clen],
                    )
                    pair.append(xs)
                xt_sbs.append(pair)

            # ---- PE warmup: the PE clock needs ~4.5us of sustained matmul
            # activity to ramp from the low pstate (213ns per 256-row f32r
            # matmul) to peak (~112ns).  Dummy matmuls fill the dead time
            # between engine start (~7us) and the weights landing (~11.5us)
            # so the fold and the main GEMM run at the ramped clock. ----
            if NWARM:
                warm_sb = cp.tile([128, 128], f32)
                nc.vector.memset(warm_sb, 0.0)
                warm_ps = psw.tile([128, C], f32, tag="w")
                for _ in range(NWARM):
                    nc.tensor.matmul(
                        warm_ps[:, 0:128], warm_sb, warm_sb,
                        start=True, stop=True,
                    )

            # ones x bias -> all-partition bias row block (PE broadcast)
            bias_bc = cp.tile([128, C], f32)
            ps_b = psw.tile([128, C], f32, tag="w")
            nc.tensor.matmul(ps_b, ones_sb, bias_sb, start=True, stop=True)
            nc.vector.tensor_copy(bias_bc, ps_b)

            # ---- fold W2T[k, p] = sum_vd Wv[vd, k] * wpt[vd, p] ----
            # (f32r consumers, so the PSUM->SBUF copy emits f32r)
            w2t_sb = cp.tile([128, 2, C], mm_dt)  # [p(k), kc, pcol]
            for kc in range(2):
                ps = psw.tile([128, C], f32, tag="w")
                for vdc in range(2):
                    nc.tensor.matmul(
                        ps,
                        wb_sb[:, vdc, kc * 128:(kc + 1) * 128],
                        wb_sb[:, 2 + vdc, :],
                        start=(vdc == 0),
                        stop=(vdc == 1),
                    )
                nc.vector.tensor_copy(w2t_sb[:, kc, :], ps)

            # ---- main GEMM: out[n, p] = sum_k xT[k, n] * W2T[k, p] + b[p] ----
            # all 8 output tiles live in one contiguous SBUF block so output
            # DMAs can cover several tiles with one fat line per partition
            ot_sb = cp.tile([128, NT, C], out_dt)
            # output DMA schedule: 2-tile chunks early (their drain hides
            # under remaining compute), single-tile chunks for the last two
            # tiles on ALTERNATE rings so the final drain is minimal
            # back-half tiles ship as singles on strictly alternating rings
            # so no output's descriptor-issue ever queues behind another
            # on the engine that the critical last tile needs
            out_sched = {1: (nc.scalar, 0), 3: (nc.sync, 2), 4: (nc.scalar, 4),
                         5: (nc.sync, 5), 6: (nc.scalar, 6), 7: (nc.sync, 7)}
            for t in range(NT):
                gi = tile_grp[t]
                xk0, xk1 = xt_sbs[gi]
                tc_off = t * 128 - groups[gi][1]
                ps = pso.tile([128, C], f32)
                nc.tensor.matmul(
                    ps, xk0[:, 0, tc_off:tc_off + 128], w2t_sb[:, 0, :],
                    start=True, stop=False,
                )
                nc.tensor.matmul(
                    ps, xk1[:, 0, tc_off:tc_off + 128], w2t_sb[:, 1, :],
                    start=False, stop=True,
                )
                nc.vector.tensor_add(ot_sb[:, t, :], ps, bias_bc)
                if t in out_sched:
                    eng, t0 = out_sched[t]
                    eng.dma_start(
                        out=out[:, t0 * C:(t + 1) * C],
                        in_=ot_sb[:, t0:t + 1, :],
                    )

    nc.compile()
    return nc


def run_sharded(inputs, trace=False, trace_cores=None):
    """Shard inputs, run on the 8 NeuronCores, gather.  Returns
    (full_output, BassKernelResults)."""
    from concourse.bass_utils import run_bass_kernel_spmd

    x = np.ascontiguousarray(np.asarray(inputs["x"], dtype=np.float32))
    w_qkv = np.ascontiguousarray(np.asarray(inputs["w_qkv"], dtype=np.float32))
    w_proj = np.ascontiguousarray(np.asarray(inputs["w_proj"], dtype=np.float32))
    b_proj = np.ascontiguousarray(np.asarray(inputs["b_proj"], dtype=np.float32))

    if "nc" not in _cache:
        _cache["nc"] = _build()
    nc = _cache["nc"]

    # host-side layout marshaling only (no FLOPs)
    xT = np.ascontiguousarray(x.reshape(ROWS, C).T)          # [256, 8192]
    wv = w_qkv[2 * C:3 * C]                                  # [256, 256]
    wpt = w_proj.T                                           # [256, 256]
    # pack wv + wpt p-major: wb[p, j, :] for j in (wv kc0, wv kc1, wpt 0, wpt 1)
    wb = np.empty((128, 4, C), dtype=np.float32)
    wb[:, 0] = wv[0:128]
    wb[:, 1] = wv[128:256]
    wb[:, 2] = wpt[0:128]
    wb[:, 3] = wpt[128:256]
    wb = np.ascontiguousarray(wb.reshape(128, 4 * C))

    in_maps = [
        {
            "xt": np.ascontiguousarray(xT[:, c * RPC:(c + 1) * RPC]),
            "wb": wb,
            "b": b_proj,
        }
        for c in range(NCORES)
    ]

    res = run_bass_kernel_spmd(
        nc,
        in_maps,
        core_ids=list(range(NCORES)),
        trace=trace,
        trace_cores=trace_cores,
    )
    # device emits [p, t, m]; undo the (t p) row permutation and widen
    # bf16 -> f32 (exact zero-extension)
    blocks = []
    for c in range(NCORES):
        arr = np.asarray(res.results[c]["out"]).reshape(128, NT, C)
        blocks.append(
            np.ascontiguousarray(arr.transpose(1, 0, 2)).reshape(RPC, C).astype(np.float32)
        )
    out = np.concatenate(blocks, axis=0)  # [8192, 256]
    return out.reshape(B, N, C), res


def kernel(x, w_qkv, w_proj, b_proj, temperature):
    out, _ = run_sharded(
        {"x": x, "w_qkv": w_qkv, "w_proj": w_proj, "b_proj": b_proj}
    )
    return out


# revision 57
# speedup vs baseline: 1.0466x; 1.0094x over previous
"""Trainium2 Bass kernel for nn_LocalitySelfAttention.

The module's attention scores get +1e9 added on the diagonal before the
softmax (torch's ``attn - diag(-1e9)``).  QK^T scores for randn inputs are
O(1), so every softmax row is an exact fp32 one-hot at the diagonal and
``attn @ v == v`` bit-exactly.  The whole module therefore reduces to

    out = x @ Wv.T @ w_proj.T + b_proj,      Wv = w_qkv[512:768]

which is a memory-bound GEMM.  The kernel shards the 8192 (B*N) rows across
the 8 NeuronCores (1024 rows each).  Each core:

  1. folds W2T[k,p] = sum_vd Wv[vd,k] * w_proj[p,vd] on the TensorEngine,
  2. computes out[n,p] = sum_k xT[k,n] * W2T[k,p] + b[p] as 8 PSUM tiles;
     the bias-add happens during the PSUM->SBUF copy on the DVE, emitting
     bf16 (the rounding is done on-device; the host only zero-extends
     bf16->f32, which is exact), halving both the copy time and the
     output HBM traffic.

All matmul operands are typed float32r end-to-end (DRAM + SBUF), which the
PE streams at half fp32's cycles-per-row; the bytes are plain fp32 and the
PSUM accumulation stays fp32 (rel err ~2e-3 vs 2e-2 tolerance).

Measured HW model this is built around:
  - exec_time = last-output-byte time + fixed overhead: the ~6us NEFF
    start is excluded by the profiler's first-useful-instruction window
    and an ~8.5us finalization tail is constant, so everything aims at
    finishing the last output DMA byte early.  First DMA bytes land a
    fixed ~2.7us after the post-barrier triggers, and the 8-core input
    phase runs at the chip HBM roofline, so the input stream itself is
    the floor.
  - dma_start runs at ~5ns/descriptor on the issuing engine and both
    HWDGE rings (SP, Act) feed the same 16 HW queues in descriptor
    ARRIVAL order, so transfers use >=2KB lines and issue order is
    arranged as: weights -> bias -> first x half (SP), then second x
    half (Act, gated on the weights' completion by a tiny Act read so
    it cannot starve the fold); outputs alternate across both rings.
  - x chunks each get their own SBUF tile (a shared buffer would
    serialize a chunk's DMA behind every reader of the previous chunk);
    the kc0/kc1 planes are separate DMAs, and the second 512 columns
    arrive as quarters so only two row-tiles of work remain after the
    final chunk's completion semaphore (which itself lands ~1-1.5us
    after the data: a DMA's 16 queue-shard completions spread out).
  - a 128-descriptor stride-0 broadcast DMA crawls (~75 B/ns) and blocks
    queue FIFOs, so the bias arrives as ONE descriptor and is broadcast
    across partitions by a one-time ones x bias matmul on the PE.

The host only moves bytes: it transposes x, packs the weight block, and
unpermutes/widens the per-core output blocks (layout + zero-extension
only, no arithmetic).
"""

import os
import sys

import numpy as np

if "/opt/trn_rl_repo" not in sys.path:
    sys.path.insert(0, "/opt/trn_rl_repo")

B, N, C = 2, 4096, 256
ROWS = B * N              # 8192
NCORES = 8
RPC = ROWS // NCORES      # 1024 rows per core
NT = RPC // 128           # 8 row-tiles of 128 per core
OUTTILES = int(os.environ.get("K_OUTTILES", "2"))  # tiles per output DMA

USE_F32R = os.environ.get("K_F32R", "1") == "1"
OUT_BF16 = os.environ.get("K_OBF16", "1") == "1"
NWARM = int(os.environ.get("K_NWARM", "0"))   # PE clock-ramp matmuls

_cache = {}


def _build():
    """Build + compile the per-core Bass program (same program, SPMD)."""
    import concourse.bacc as bacc
    import concourse.bass as bass
    import concourse.mybir as mybir
    import concourse.tile as tile

    f32 = mybir.dt.float32
    mm_dt = mybir.dt.float32r if USE_F32R else f32
    out_dt = mybir.dt.bfloat16 if OUT_BF16 else f32

    nc = bacc.Bacc(
        "TRN2",
        target_bir_lowering=False,
        debug=False,
        num_devices=NCORES,
    )

    # All matmul inputs are typed f32r in DRAM too: the BIR verifier
    # requires every producer feeding an FP32r matmult to emit f32r, and
    # a DMA from an f32r DRAM tensor satisfies it (bytes are plain fp32).
    xt_d = nc.dram_tensor("xt", [C, RPC], mm_dt, kind="ExternalInput")
    wb_d = nc.dram_tensor("wb", [128, 4 * C], mm_dt, kind="ExternalInput")
    b_d = nc.dram_tensor("b", [C], f32, kind="ExternalInput")
    # output laid out [p, t, m] so multi-tile DMAs get fat contiguous lines;
    # the host undoes the (t p) permutation
    out_d = nc.dram_tensor("out", [128, NT * C], out_dt, kind="ExternalOutput")

    xt = xt_d.ap()
    wb = wb_d.ap()
    b = b_d.ap()
    out = out_d.ap()

    with tile.TileContext(nc) as tc:
        with (
            tc.tile_pool(name="const", bufs=1) as cp,
            tc.tile_pool(name="psw", bufs=3, space="PSUM") as psw,
            tc.tile_pool(name="pso", bufs=5, space="PSUM") as pso,
        ):
            # Both HWDGE rings (SP and Act) feed the SAME 16 HW queues in
            # descriptor-ARRIVAL order, so completion order is controlled
            # entirely by when each engine writes its descriptors.  Wanted
            # order: wb (fold) -> early x chunks -> late x chunks.

            # ---- weights first on SP: one DMA, 128 x 4KB lines ----
            # wb_sb[p, 0:2, k] = Wv[vdc*128+p, k]; [p, 2:4, q] = WprojT[vdc*128+p, q]
            wb_sb = cp.tile([128, 4, C], mm_dt)
            nc.sync.dma_start(out=wb_sb, in_=wb.rearrange("p (j k) -> p j k", j=4))

            # bias: ONE descriptor to a single partition (a 128-descriptor
            # stride-0 broadcast DMA crawls at ~75 B/ns and blocks every
            # queue FIFO behind it), then a one-time ones x bias matmul
            # broadcasts across partitions via the PE
            bias_sb = cp.tile([1, C], f32)
            nc.sync.dma_start(out=bias_sb, in_=b.rearrange("(o c) -> o c", o=1))
            ones_sb = cp.tile([1, 128], f32)
            nc.vector.memset(ones_sb, 1.0)

            # tiny Act-engine read of bias_sb: forces Act to wait until the
            # bias descriptor (queued right BEHIND all of wb's) completes
            # before issuing the late x chunks, so their descriptors arrive
            # after wb's and the fold is never starved behind x traffic.
            # bias's single-descriptor semaphore lands ~0.8us earlier than
            # wb's own 16-shard completion semaphore (shard completions
            # spread over ~1us), so it is the cheaper proxy for the same
            # ordering fact.
            wgate = cp.tile([1, 16], f32)
            nc.scalar.copy(wgate, bias_sb[0:1, 0:16])

            # ---- x^T slice, k-major [k=256, n=1024], chunked by column
            # group x kc.  Chunks complete in arrival order: the first 512
            # columns (tiles 0-3) issue on SP right behind wb; the second
            # half issues on Act behind the wb gate as QUARTERS (256 cols,
            # i.e. 2 tiles each) so after the very last chunk lands only
            # two tiles' matmuls remain.  Distinct tag per chunk — one
            # shared buffer would serialize chunk DMAs behind readers. ----
            xt_v = xt.rearrange("(kc p) n -> p kc n", p=128)
            # (engine, col_start, col_len) per chunk group; each group is
            # a kc0+kc1 pair of DMAs
            groups = [
                (nc.sync, 0, 512),
                (nc.scalar, 512, 256),
                (nc.scalar, 768, 256),
            ]
            tile_grp = [0, 0, 0, 0, 1, 1, 2, 2]   # row-tile -> chunk group
            xt_sbs = []      # [group][kc] -> tile [128, 1, col_len]
            for gi, (eng, c0, clen) in enumerate(groups):
                pair = []
                for kc in range(2):
                    xs = cp.tile([128, 1, clen], mm_dt, tag=f"xchunk{gi}_{kc}")
                    eng.dma_start(
                        out=xs,
                        in_=xt_v[:, kc:kc + 1, c0:c0 + clen],
                    )
                    pair.append(xs)
                xt_sbs.append(pair)

            # ---- PE warmup: the PE clock needs ~4.5us of sustained matmul
            # activity to ramp from the low pstate (213ns per 256-row f32r
            # matmul) to peak (~112ns).  Dummy matmuls fill the dead time
            # between engine start (~7us) and the weights landing (~11.5us)
            # so the fold and the main GEMM run at the ramped clock. ----
            if NWARM:
                warm_sb = cp.tile([128, 128], f32)
                nc.vector.memset(warm_sb, 0.0)
                warm_ps = psw.tile([128, C], f32, tag="w")
                for _ in range(NWARM):
                    nc.tensor.matmul(
                        warm_ps[:, 0:128], warm_sb, warm_sb,
                        start=True, stop=True,
                    )

            # ones x bias -> all-partition bias row block (PE broadcast)
            bias_bc = cp.tile([128, C], f32)
            ps_b = psw.tile([128, C], f32, tag="w")
            nc.tensor.matmul(ps_b, ones_sb, bias_sb, start=True, stop=True)
            nc.vector.tensor_copy(bias_bc, ps_b)

            # ---- fold W2T[k, p] = sum_vd Wv[vd, k] * wpt[vd, p] ----
            # (f32r consumers, so the PSUM->SBUF copy emits f32r)
            w2t_sb = cp.tile([128, 2, C], mm_dt)  # [p(k), kc, pcol]
            for kc in range(2):
                ps = psw.tile([128, C], f32, tag="w")
                for vdc in range(2):
                    nc.tensor.matmul(
                        ps,
                        wb_sb[:, vdc, kc * 128:(kc + 1) * 128],
                        wb_sb[:, 2 + vdc, :],
                        start=(vdc == 0),
                        stop=(vdc == 1),
                    )
                nc.vector.tensor_copy(w2t_sb[:, kc, :], ps)

            # ---- main GEMM: out[n, p] = sum_k xT[k, n] * W2T[k, p] + b[p] ----
            # all 8 output tiles live in one contiguous SBUF block so output
            # DMAs can cover several tiles with one fat line per partition
            ot_sb = cp.tile([128, NT, C], out_dt)
            # output DMA schedule: 2-tile chunks early (their drain hides
            # under remaining compute), single-tile chunks for the last two
            # tiles on ALTERNATE rings so the final drain is minimal
            out_sched = {1: (nc.scalar, 0), 3: (nc.sync, 2), 5: (nc.scalar, 4),
                         6: (nc.sync, 6), 7: (nc.scalar, 7)}
            for t in range(NT):
                gi = tile_grp[t]
                xk0, xk1 = xt_sbs[gi]
                tc_off = t * 128 - groups[gi][1]
                ps = pso.tile([128, C], f32)
                nc.tensor.matmul(
                    ps, xk0[:, 0, tc_off:tc_off + 128], w2t_sb[:, 0, :],
                    start=True, stop=False,
                )
                nc.tensor.matmul(
                    ps, xk1[:, 0, tc_off:tc_off + 128], w2t_sb[:, 1, :],
                    start=False, stop=True,
                )
                nc.vector.tensor_add(ot_sb[:, t, :], ps, bias_bc)
                if t in out_sched:
                    eng, t0 = out_sched[t]
                    eng.dma_start(
                        out=out[:, t0 * C:(t + 1) * C],
                        in_=ot_sb[:, t0:t + 1, :],
                    )

    nc.compile()
    return nc


def run_sharded(inputs, trace=False, trace_cores=None):
    """Shard inputs, run on the 8 NeuronCores, gather.  Returns
    (full_output, BassKernelResults)."""
    from concourse.bass_utils import run_bass_kernel_spmd

    x = np.ascontiguousarray(np.asarray(inputs["x"], dtype=np.float32))
    w_qkv = np.ascontiguousarray(np.asarray(inputs["w_qkv"], dtype=np.float32))
    w_proj = np.ascontiguousarray(np.asarray(inputs["w_proj"], dtype=np.float32))
    b_proj = np.ascontiguousarray(np.asarray(inputs["b_proj"], dtype=np.float32))

    if "nc" not in _cache:
        _cache["nc"] = _build()
    nc = _cache["nc"]

    # host-side layout marshaling only (no FLOPs)
    xT = np.ascontiguousarray(x.reshape(ROWS, C).T)          # [256, 8192]
    wv = w_qkv[2 * C:3 * C]                                  # [256, 256]
    wpt = w_proj.T                                           # [256, 256]
    # pack wv + wpt p-major: wb[p, j, :] for j in (wv kc0, wv kc1, wpt 0, wpt 1)
    wb = np.empty((128, 4, C), dtype=np.float32)
    wb[:, 0] = wv[0:128]
    wb[:, 1] = wv[128:256]
    wb[:, 2] = wpt[0:128]
    wb[:, 3] = wpt[128:256]
    wb = np.ascontiguousarray(wb.reshape(128, 4 * C))

    in_maps = [
        {
            "xt": np.ascontiguousarray(xT[:, c * RPC:(c + 1) * RPC]),
            "wb": wb,
            "b": b_proj,
        }
        for c in range(NCORES)
    ]

    res = run_bass_kernel_spmd(
        nc,
        in_maps,
        core_ids=list(range(NCORES)),
        trace=trace,
        trace_cores=trace_cores,
    )
    # device emits [p, t, m]; undo the (t p) row permutation and widen
    # bf16 -> f32 (exact zero-extension)
    blocks = []
    for c in range(NCORES):
        arr = np.asarray(res.results[c]["out"]).reshape(128, NT, C)
        blocks.append(
            np.ascontiguousarray(arr.transpose(1, 0, 2)).reshape(RPC, C).astype(np.float32)
        )
    out = np.concatenate(blocks, axis=0)  # [8192, 256]
    return out.reshape(B, N, C), res


def kernel(x, w_qkv, w_proj, b_proj, temperature):
    out, _ = run_sharded(
        {"x": x, "w_qkv": w_qkv, "w_proj": w_proj, "b_proj": b_proj}
    )
    return out
